# revision 27
# baseline (speedup 1.0000x reference)
"""BVRNN forward kernel for Trainium2 (Bass/Tile), self-contained.

Strategy (v1):
- Feature-major layout on chip: activations stored as [feat_part, batch_free]
  with batch B=64 as the free dim. Weights live in SBUF as lhsT [K, N] tiles;
  each matmul contracts one 128-wide K chunk (weights stationary), streaming
  the 64 batch columns. No transposes needed anywhere.
- phi_x stack is precomputed for all (t, b) in a wide pass (full PE rate),
  stored to DRAM shifted by +1 (see below), then the T=500 recurrence runs
  in a dynamic For_i loop on one core.
- ELU trick: elu(v) = relu(v) + exp(min(v, 0)) - 1 and exp(min(v,0)) =
  min(exp(v), 1). We store a' = elu(v)+1 = relu(v+b) + min(exp(v+b), 1)
  (2 fused tensor_scalar ops + 1 Exp activation + 1 add per 128-chunk) and
  fold the -1 into the next layer's bias: b' = b - rowsum(W over shifted-in
  columns).
- round(enc) via Sign(enc - 0.5) mapped through 0.5*s + 0.5 (exact for
  enc != 0.5).
- kld accumulated on-chip; reduced at the end (ones-matmul over partitions).
"""

import numpy as np

import concourse.bass as bass
import concourse.tile as tile
from concourse import bacc, mybir
from concourse.bass import ts, ds
from concourse.bass_utils import run_bass_kernel_spmd

F32 = mybir.dt.float32
AFT = mybir.ActivationFunctionType
ALU = mybir.AluOpType

B = 64
T = 500
X = 80
H = 512
Z = 64
EPS = 1e-4
COLS = B * T  # 32000, column order is (t, b): col = t*B + b


def _cdiv(a, b):
    return (a + b - 1) // b


def build_kernel(T=T, CB=512):
    COLS = B * T
    nc = bacc.Bacc()

    # ---- I/O -------------------------------------------------------------
    yT = nc.dram_tensor("yT", [X, COLS], F32, kind="ExternalInput")

    wspecs = {
        # name: (K, N)
        "pxw1": (X, H), "pxw2": (H, H), "pxw3": (H, H),
        "ew1": (2 * H, H), "ew2": (H, H), "ew3": (H, Z),
        "prw1": (H, H), "prw2": (H, H), "prw3": (H, Z),
        "zw1": (Z, H), "zw2": (H, H), "zw3": (H, H),
        "dw1": (2 * H, H), "dw2": (H, H), "dw3": (H, H), "dw4": (H, X),
        "gwi": (2 * H, 3 * H), "gwh": (H, 3 * H),
    }
    wd = {}
    bd = {}
    for name, (K, N) in wspecs.items():
        wd[name] = nc.dram_tensor("w_" + name, [K, N], F32, kind="ExternalInput")
        bd[name] = nc.dram_tensor("b_" + name, [N, 1], F32, kind="ExternalInput")

    decsT = nc.dram_tensor("decsT", [T * X, B], F32, kind="ExternalOutput")
    kld_out = nc.dram_tensor("kld_out", [1, 1], F32, kind="ExternalOutput")

    # internal scratch: precomputed (elu+1)-shifted phi_x, feature-major
    # row index = t*128 + p, dims [t*p, kchunk, b]
    pxT = nc.dram_tensor("pxT", [T * 128, H // 128, B], F32)

    from contextlib import ExitStack
    with tile.TileContext(nc) as tc, ExitStack() as stack:
        wsb = {}
        bsb = {}

        def load_weights(pool, names):
            for name in names:
                K, N = wspecs[name]
                pK = min(K, 128)
                nK = _cdiv(K, 128)
                t_w = pool.tile([pK, nK, N], F32, tag="w_" + name)
                nc.gpsimd.dma_start(
                    out=t_w,
                    in_=wd[name][:].rearrange("(nk p) n -> p nk n", p=pK))
                wsb[name] = t_w
                pN = min(N, 128)
                nN = _cdiv(N, 128)
                t_b = pool.tile([pN, nN], F32, tag="b_" + name)
                nc.gpsimd.dma_start(
                    out=t_b,
                    in_=bd[name][:].rearrange("(nn p) one -> p (nn one)",
                                              p=pN))
                bsb[name] = t_b

        # ================== helpers =======================================
        def layer_matmul(psum_t, wname, rhs_of_k, nN_cols=None, cols=B):
            """Accumulate full layer into psum_t [pN, nN, cols].

            rhs_of_k: function k -> AP [128(or pK), cols]
            """
            K, N = wspecs[wname]
            pK = min(K, 128)
            nK = _cdiv(K, 128)
            pN = min(N, 128)
            nN = _cdiv(N, 128) if nN_cols is None else nN_cols
            w = wsb[wname]
            for n in range(nN):
                n0 = n * 128
                nn = min(128, N - n0)
                for k in range(nK):
                    nc.tensor.matmul(
                        psum_t[:nn, n, :],
                        w[:pK, k, n0:n0 + nn],
                        rhs_of_k(k),
                        start=(k == 0),
                        stop=(k == nK - 1),
                    )

        def elu_evict(pool, psum_t, wname, cols=B, tag=None, out=None):
            """PSUM -> SBUF with a' = relu(v+b) + min(exp(v+b), 1)."""
            K, N = wspecs[wname]
            pN = min(N, 128)
            nN = _cdiv(N, 128)
            bias = bsb[wname]
            if out is None:
                out = pool.tile([pN, nN, cols], F32, tag=tag or ("a_" + wname))
            for n in range(nN):
                b_col = bias[:pN, n:n + 1]
                e = pool.tile([pN, cols], F32, tag="elu_e")
                nc.scalar.activation(e, psum_t[:pN, n, :], AFT.Exp, bias=b_col)
                em = pool.tile([pN, cols], F32, tag="elu_em")
                nc.vector.tensor_scalar(em, e, 1.0, None, ALU.min)
                r = pool.tile([pN, cols], F32, tag="elu_r")
                nc.vector.tensor_scalar(r, psum_t[:pN, n, :], b_col, 0.0,
                                        ALU.add, ALU.max)
                nc.vector.tensor_add(out[:pN, n, :], r, em)
            return out

        # ================== phase 1: precompute phi_x =====================
        nblocks = _cdiv(COLS, CB)
        with tc.tile_pool(name="pxw", bufs=1) as pxwpool, \
                tc.tile_pool(name="pre_sbuf", bufs=2) as pp, \
                tc.tile_pool(name="pre_psum", bufs=2, space="PSUM") as pps:
            load_weights(pxwpool, ["pxw1", "pxw2", "pxw3"])

            for blk in range(nblocks):
                c0 = blk * CB
                cols = min(CB, COLS - c0)
                ym = pp.tile([X, cols], F32, tag="ym")
                nc.gpsimd.dma_start(out=ym, in_=yT[:, c0:c0 + cols])

                ps1 = pps.tile([128, H // 128, cols], F32, tag="pp")
                layer_matmul(ps1, "pxw1", lambda k: ym, cols=cols)
                a1 = elu_evict(pp, ps1, "pxw1", cols=cols, tag="a1")

                ps2 = pps.tile([128, H // 128, cols], F32, tag="pp")
                layer_matmul(ps2, "pxw2", lambda k: a1[:, k, :], cols=cols)
                a2 = elu_evict(pp, ps2, "pxw2", cols=cols, tag="a2")

                ps3 = pps.tile([128, H // 128, cols], F32, tag="pp")
                layer_matmul(ps3, "pxw3", lambda k: a2[:, k, :], cols=cols)
                a3 = elu_evict(pp, ps3, "pxw3", cols=cols, tag="a3")

                nsteps = cols // B
                t0 = c0 // B
                nc.gpsimd.dma_start(
                    out=pxT[t0 * 128:(t0 + nsteps) * 128, :, :].rearrange(
                        "(t p) k b -> p k t b", p=128),
                    in_=a3[:, :, :].rearrange("p k (t b) -> p k t b", b=B))

        # ================== phase 2: the scan =============================
        nH = H // 128
        wpool = stack.enter_context(tc.tile_pool(name="weights", bufs=1))
        load_weights(wpool, [n for n in wspecs if not n.startswith("pxw")])
        with tc.tile_pool(name="state", bufs=1) as st, \
                tc.tile_pool(name="act", bufs=2) as ap, \
                tc.tile_pool(name="mm", bufs=5, space="PSUM") as mp, \
                tc.tile_pool(name="sm", bufs=1, space="PSUM") as sp:
            h = st.tile([128, nH, B], F32)
            nc.vector.memset(h, 0.0)
            kld_acc = st.tile([Z, B], F32)
            nc.vector.memset(kld_acc, 0.0)
            ones_col = st.tile([Z, 1], F32)
            nc.vector.memset(ones_col, 1.0)
            cm05 = st.tile([128, 1], F32)
            nc.vector.memset(cm05, -0.5)
            ceps = st.tile([128, 1], F32)
            nc.vector.memset(ceps, EPS)
            c1eps = st.tile([128, 1], F32)
            nc.vector.memset(c1eps, 1.0 + EPS)

            with tc.For_i(0, T, 1) as iv:
                px_t = ap.tile([128, nH, B], F32, tag="px_t")
                nc.gpsimd.dma_start(out=px_t, in_=pxT[ts(iv, 128)])

                def cat_px_h(k):
                    return px_t[:, k, :] if k < nH else h[:, k - nH, :]

                # enc
                ps = mp.tile([128, nH, B], F32, tag="mm")
                layer_matmul(ps, "ew1", cat_px_h)
                e1 = elu_evict(ap, ps, "ew1")
                ps = mp.tile([128, nH, B], F32, tag="mm")
                layer_matmul(ps, "ew2", lambda k: e1[:, k, :])
                e2 = elu_evict(ap, ps, "ew2")
                ps_enc = sp.tile([Z, 1, B], F32, tag="sm")
                layer_matmul(ps_enc, "ew3", lambda k: e2[:, k, :])
                enc = ap.tile([Z, B], F32, tag="enc")
                nc.scalar.activation(enc, ps_enc[:Z, 0, :], AFT.Sigmoid,
                                     bias=bsb["ew3"][:Z, 0:1])

                # prior
                ps = mp.tile([128, nH, B], F32, tag="mm")
                layer_matmul(ps, "prw1", lambda k: h[:, k, :])
                p1 = elu_evict(ap, ps, "prw1")
                ps = mp.tile([128, nH, B], F32, tag="mm")
                layer_matmul(ps, "prw2", lambda k: p1[:, k, :])
                p2 = elu_evict(ap, ps, "prw2")
                ps_pr = sp.tile([Z, 1, B], F32, tag="sm")
                layer_matmul(ps_pr, "prw3", lambda k: p2[:, k, :])
                prior = ap.tile([Z, B], F32, tag="prior")
                nc.scalar.activation(prior, ps_pr[:Z, 0, :], AFT.Sigmoid,
                                     bias=bsb["prw3"][:Z, 0:1])

                # z = round(enc) via sign
                zs = ap.tile([Z, B], F32, tag="zs")
                nc.scalar.activation(zs, enc, AFT.Sign, bias=cm05[:Z, 0:1])
                z_t = ap.tile([Z, B], F32, tag="z_t")
                nc.vector.tensor_scalar(z_t, zs, 0.5, 0.5, ALU.mult, ALU.add)

                # phi_z
                ps = mp.tile([128, nH, B], F32, tag="mm")
                layer_matmul(ps, "zw1", lambda k: z_t)
                q1 = elu_evict(ap, ps, "zw1")
                ps = mp.tile([128, nH, B], F32, tag="mm")
                layer_matmul(ps, "zw2", lambda k: q1[:, k, :])
                q2 = elu_evict(ap, ps, "zw2")
                ps = mp.tile([128, nH, B], F32, tag="mm")
                layer_matmul(ps, "zw3", lambda k: q2[:, k, :])
                phz = elu_evict(ap, ps, "zw3")

                def cat_phz_h(k):
                    return phz[:, k, :] if k < nH else h[:, k - nH, :]

                # dec
                ps = mp.tile([128, nH, B], F32, tag="mm")
                layer_matmul(ps, "dw1", cat_phz_h)
                d1 = elu_evict(ap, ps, "dw1")
                ps = mp.tile([128, nH, B], F32, tag="mm")
                layer_matmul(ps, "dw2", lambda k: d1[:, k, :])
                d2 = elu_evict(ap, ps, "dw2")
                ps = mp.tile([128, nH, B], F32, tag="mm")
                layer_matmul(ps, "dw3", lambda k: d2[:, k, :])
                d3 = elu_evict(ap, ps, "dw3")
                ps_d4 = sp.tile([X, 1, B], F32, tag="smx")
                layer_matmul(ps_d4, "dw4", lambda k: d3[:, k, :])
                dec_sb = ap.tile([X, B], F32, tag="dec")
                nc.scalar.activation(dec_sb, ps_d4[:X, 0, :], AFT.Identity,
                                     bias=bsb["dw4"][:X, 0:1])
                nc.gpsimd.dma_start(out=decsT[ts(iv, X)], in_=dec_sb)

                # gru gates; gwi consumes (px|phz), gwh consumes h.
                def cat_px_phz(k):
                    return px_t[:, k, :] if k < nH else phz[:, k - nH, :]

                gK, gN = wspecs["gwi"]
                r_sb = ap.tile([128, nH, B], F32, tag="r_sb")
                zz_sb = ap.tile([128, nH, B], F32, tag="zz_sb")
                for g in range(3):
                    ps_gi = mp.tile([128, nH, B], F32, tag="mm")
                    if g == 0 or g == 1:
                        # r/z gates: PE accumulates gi and gh into one group
                        for n in range(nH):
                            n0 = g * H + n * 128
                            for k in range(2 * nH):
                                nc.tensor.matmul(
                                    ps_gi[:, n, :],
                                    wsb["gwi"][:, k, n0:n0 + 128],
                                    cat_px_phz(k),
                                    start=(k == 0), stop=False)
                            for k in range(nH):
                                nc.tensor.matmul(
                                    ps_gi[:, n, :],
                                    wsb["gwh"][:, k, n0:n0 + 128],
                                    h[:, k, :],
                                    start=False, stop=(k == nH - 1))
                    else:
                        ps_gh = mp.tile([128, nH, B], F32, tag="mm")
                        for n in range(nH):
                            n0 = g * H + n * 128
                            for k in range(2 * nH):
                                nc.tensor.matmul(
                                    ps_gi[:, n, :],
                                    wsb["gwi"][:, k, n0:n0 + 128],
                                    cat_px_phz(k),
                                    start=(k == 0), stop=(k == 2 * nH - 1))
                            for k in range(nH):
                                nc.tensor.matmul(
                                    ps_gh[:, n, :],
                                    wsb["gwh"][:, k, n0:n0 + 128],
                                    h[:, k, :],
                                    start=(k == 0), stop=(k == nH - 1))
                    for n in range(nH):
                        nchunk = g * nH + n
                        bi_col = bsb["gwi"][:, nchunk:nchunk + 1]
                        bh_col = bsb["gwh"][:, nchunk:nchunk + 1]
                        if g == 0 or g == 1:
                            # r/z = sigmoid(gi + gh + bi + bh); bh folded into
                            # bi on host (b_gwi = b_ih + b_hh for r,z gates)
                            dst = r_sb if g == 0 else zz_sb
                            nc.scalar.activation(dst[:, n, :], ps_gi[:, n, :],
                                                 AFT.Sigmoid, bias=bi_col)
                        else:
                            # n = tanh(gi + bi + r*(gh + bh))
                            hn = ap.tile([128, B], F32, tag="gt")
                            nc.scalar.activation(hn, ps_gh[:, n, :],
                                                 AFT.Identity, bias=bh_col)
                            rm = ap.tile([128, B], F32, tag="gt2")
                            nc.vector.tensor_mul(rm, r_sb[:, n, :], hn)
                            t2 = ap.tile([128, B], F32, tag="gt3")
                            nc.vector.tensor_add(t2, rm, ps_gi[:, n, :])
                            nt = ap.tile([128, B], F32, tag="nt")
                            nc.scalar.activation(nt, t2, AFT.Tanh, bias=bi_col)
                            # h = (1-z)*n + z*h = n + z*(h-n)
                            dmn = ap.tile([128, B], F32, tag="gt")
                            nc.vector.tensor_sub(dmn, h[:, n, :], nt)
                            zm = ap.tile([128, B], F32, tag="gt2")
                            nc.vector.tensor_mul(zm, zz_sb[:, n, :], dmn)
                            nc.vector.tensor_add(h[:, n, :], nt, zm)

                # kld element accumulation
                la = ap.tile([Z, B], F32, tag="kl1")
                nc.scalar.activation(la, enc, AFT.Ln, bias=ceps[:Z, 0:1])
                lb = ap.tile([Z, B], F32, tag="kl2")
                nc.scalar.activation(lb, prior, AFT.Ln, bias=ceps[:Z, 0:1])
                lc = ap.tile([Z, B], F32, tag="kl3")
                nc.scalar.activation(lc, enc, AFT.Ln, bias=c1eps[:Z, 0:1],
                                     scale=-1.0)
                ld = ap.tile([Z, B], F32, tag="kl4")
                nc.scalar.activation(ld, prior, AFT.Ln, bias=c1eps[:Z, 0:1],
                                     scale=-1.0)
                d1k = ap.tile([Z, B], F32, tag="kl1")
                nc.vector.tensor_sub(d1k, la, lb)
                d2k = ap.tile([Z, B], F32, tag="kl2")
                nc.vector.tensor_sub(d2k, lc, ld)
                d3k = ap.tile([Z, B], F32, tag="kl3")
                nc.vector.tensor_sub(d3k, d1k, d2k)
                m1 = ap.tile([Z, B], F32, tag="kl4")
                nc.vector.tensor_mul(m1, enc, d3k)
                m2 = ap.tile([Z, B], F32, tag="kl1")
                nc.vector.tensor_add(m2, m1, d2k)
                nc.vector.tensor_add(kld_acc, kld_acc, m2)

            # final kld reduction: sum over batch (free) then features (part)
            red = st.tile([Z, 1], F32)
            nc.vector.reduce_sum(red, kld_acc, axis=mybir.AxisListType.X)
            kps = sp.tile([1, 1, 1], F32, tag="kld")
            nc.tensor.matmul(kps[:1, 0, :], ones_col, red, start=True,
                             stop=True)
            kld_sb = st.tile([1, 1], F32)
            nc.vector.tensor_scalar(kld_sb, kps[:1, 0, :], 1.0 / (T * B),
                                    None, ALU.mult)
            nc.gpsimd.dma_start(out=kld_out[:], in_=kld_sb)

    nc.compile()
    return nc


_CACHED = {}


def kernel(y, mean_mel, std_mel, params, p_use_gen, greedy, varBitrate):
    y = np.asarray(y, np.float32)
    mean_mel = np.asarray(mean_mel, np.float32)
    std_mel = np.asarray(std_mel, np.float32)

    def W(p):
        return np.asarray(p["W"], np.float32)

    def bias(p):
        return np.asarray(p["b"], np.float32)

    P = params
    ins = {}
    # yT in (x, (t b)) order
    ins["yT"] = np.ascontiguousarray(
        y.transpose(2, 1, 0).reshape(X, COLS))

    def put(name, Wm, bv, shifted_cols=None):
        """Store transposed weight and bias corrected for (elu+1)-shifted
        inputs: b' = b - sum over shifted input cols of W."""
        Wm = np.asarray(Wm, np.float32)
        bv = np.asarray(bv, np.float32).copy()
        if shifted_cols is not None:
            bv -= Wm[:, shifted_cols].sum(axis=1)
        ins["w_" + name] = np.ascontiguousarray(Wm.T)
        ins["b_" + name] = bv.reshape(-1, 1)

    sl_all = slice(None)
    px = P["phi_x"]
    # fold input normalization into the first layer: W1' = W1 * (1/std),
    # b1' = b1 - W1 @ (mean/std). Exact when std=1, mean=0.
    W1 = W(px[0]) * (1.0 / std_mel)[None, :]
    b1 = bias(px[0]) - W(px[0]) @ (mean_mel / std_mel)
    put("pxw1", W1, b1)
    put("pxw2", W(px[1]), bias(px[1]), sl_all)
    put("pxw3", W(px[2]), bias(px[2]), sl_all)
    enc = P["enc"]
    put("ew1", W(enc[0]), bias(enc[0]), slice(0, H))   # px part shifted
    put("ew2", W(enc[1]), bias(enc[1]), sl_all)
    put("ew3", W(enc[2]), bias(enc[2]), sl_all)
    pr = P["prior"]
    put("prw1", W(pr[0]), bias(pr[0]))                 # h input, exact
    put("prw2", W(pr[1]), bias(pr[1]), sl_all)
    put("prw3", W(pr[2]), bias(pr[2]), sl_all)
    pz = P["phi_z"]
    put("zw1", W(pz[0]), bias(pz[0]))                  # z input, exact
    put("zw2", W(pz[1]), bias(pz[1]), sl_all)
    put("zw3", W(pz[2]), bias(pz[2]), sl_all)
    dc = P["dec"]
    put("dw1", W(dc[0]), bias(dc[0]), slice(0, H))     # phz part shifted
    put("dw2", W(dc[1]), bias(dc[1]), sl_all)
    put("dw3", W(dc[2]), bias(dc[2]), sl_all)
    put("dw4", W(dc[3]), bias(dc[3]), sl_all)
    g = P["gru"]
    Wih = np.asarray(g["W_ih"], np.float32)
    Whh = np.asarray(g["W_hh"], np.float32)
    b_ih = np.asarray(g["b_ih"], np.float32)
    b_hh = np.asarray(g["b_hh"], np.float32)
    # gwi input (px|phz) both shifted; r/z gates: fold b_hh into b_ih.
    bi = b_ih - Wih.sum(axis=1)
    bi2 = bi.copy()
    bi2[0:2 * H] += b_hh[0:2 * H]
    ins["w_gwi"] = np.ascontiguousarray(Wih.T)
    ins["b_gwi"] = bi2.reshape(-1, 1)
    ins["w_gwh"] = np.ascontiguousarray(Whh.T)
    ins["b_gwh"] = b_hh.reshape(-1, 1)

    key = "nc"
    if key not in _CACHED:
        _CACHED[key] = build_kernel()
    nc = _CACHED[key]

    res = run_bass_kernel_spmd(nc, [ins], [0])
    out = res.results[0]
    decsT = out["decsT"].reshape(T, X, B)
    decs = np.ascontiguousarray(decsT.transpose(2, 0, 1))
    kld = np.float32(out["kld_out"].reshape(())[()])
    return decs, kld
